# revision 8
# baseline (speedup 1.0000x reference)
"""LM head log_softmax kernel for 8 Trainium2 NeuronCores.

Computes log_softmax(h @ W^T) for h [2,2048,1024] f32, W [50257,1024] f32.

Strategy (tensor parallel over vocab):
  - W sharded along vocab across 8 cores (6284 padded cols each; 8*6284 =
    50272, 15 cols of zero padding on the last core only).
  - The tensor engine is power-throttled to ~0.516 ns/column sustained for
    fp8 DoubleRow matmuls (measured; LDWEIGHTS fully overlaps), so the PE
    stream is the pacer at ~13.0us per 128-token m-tile. Everything else
    is scheduled to never stall it:
      * per m-tile, 7 column groups (6x1024 + 140) rotate over 4 PSUM
        tiles of 1024 f32 (2 banks each) - each tile's drain window is 3
        full groups (~6us) vs a ~0.9us drain, so the PE never waits on a
        bank.
      * all drains (psum f32 -> staged bf16, folding the 1/W_SCALE
        descale) + the pass2 subtract + small reductions live on the DVE
        (~55% busy); exp+accum (row sums) + per-block Ln live on the
        scalar engine (~65% busy), reading the staged bf16 copy so psum
        is released by the single DVE read.
  - Per block of m-tiles, row sums are all-reduced across the 8 cores
    (tiny payload, ~30us latency each, serialized on one CC stream).
    Blocks [6,6,6,6,5,3]: spacing > 30us so collectives never queue on
    the stream, and the 3-m-tile final block keeps the tail at
    ~(sum lag + 30us CC + 3 pass2 tiles).
  - Pass 2 per m-tile: one in-place 4x-mode DVE tensor_scalar subtract
    (stage -= lse) + bf16 DMA out, spread at most 2 per m-tile seam with
    >= 1 m-tile of lag (hides CC latency without bursting DVE/DMA).
  - W is prefetched in vocab-chunk order (all 8 k-tiles of group 0's
    columns first), so the first matmul starts ~4us in instead of
    waiting for the full 6.4MB W stream.
  - Output travels as bf16 (halves the HBM write); host casts to f32.
  - Vocab padding (zero W rows -> logit 0 -> exp 1) is corrected by a
    host-supplied additive adjustment to the local row sums (-n_pad on
    the last core), exact since exp(0) == 1.

Host side: transposes h and the W shard to K-major (fp8), launches the SPMD
kernel via run_bass_kernel_spmd on cores 0-7, concatenates the per-core
[4096, 6284] bf16 outputs along vocab, slices off the padding, casts to f32.
"""

import os

# ASAP scheduling: per-engine execution follows emission order (the legacy
# CoreSim-timed scheduler mis-models fp8 matmul cost 2x-fast and collectives
# as instant, so it reorders pass2/drain/trigger work destructively).
os.environ.setdefault("TILE_SCHEDULER", "asap")

import numpy as np

import concourse.bacc as bacc
import concourse.mybir as mybir
import concourse.tile as tile
from concourse.bass_utils import run_bass_kernel_spmd

N_CORES = 8
B, S, D = 2, 2048, 1024
T = B * S                      # 4096 tokens
V = 50257
VC = 6284                      # per-core padded vocab shard (8*6284 = 50272)
P = 128                        # SBUF partitions
K_TILES = D // P               # 8
K_PAIRS = K_TILES // 2         # 4 (fp8 DoubleRow handles 2 k-tiles at once)
M_TILES = T // P               # 32
GW = 1024                      # column-group width
GROUPS = [(g * GW, min(GW, VC - g * GW)) for g in range((VC + GW - 1) // GW)]
# small blocks: one AllReduce per 3 m-tiles (~39us) keeps the CC stream
# ~75% busy, bounds lse latency to ~33.5us after block end, and acts as a
# recurring cross-core barrier that keeps the 8 cores in lockstep
BLOCKS = [3] * 10 + [2]        # sums to 32 m-tiles

BF16 = mybir.dt.bfloat16
F32 = mybir.dt.float32
FP8 = mybir.dt.float8e4
NP_FP8 = mybir.dt.np(mybir.dt.float8e4)
NP_BF16 = mybir.dt.np(mybir.dt.bfloat16)
W_SCALE = 32.0

# results of the last run_bass_kernel_spmd call (for test harness inspection)
LAST_RESULT = None


def build_nc():
    nc = bacc.Bacc(
        "TRN2",
        target_bir_lowering=False,
        debug=False,
        num_devices=N_CORES,
    )
    hT = nc.dram_tensor("hT", [D, T], FP8, kind="ExternalInput").ap()
    wT = nc.dram_tensor("wT", [D, VC], FP8, kind="ExternalInput").ap()
    adj = nc.dram_tensor("adj", [P, 1], F32, kind="ExternalInput").ap()
    out = nc.dram_tensor("out", [T, VC], BF16, kind="ExternalOutput").ap()

    hT_r = hT.rearrange("(k p) m -> p k m", p=P)
    wT_r = wT.rearrange("(k p) n -> p k n", p=P)

    with tile.TileContext(nc) as tc:
        with (
            tc.tile_pool(name="singles", bufs=1) as singles,
            tc.tile_pool(name="hts", bufs=6) as hts_pool,
            tc.tile_pool(name="psum", bufs=4, space="PSUM") as psum_pool,
            tc.tile_pool(name="stage", bufs=10) as stage_pool,
            tc.tile_pool(name="scratch", bufs=2) as scratch_pool,
            tc.tile_pool(name="stats", bufs=12) as stats_pool,
            tc.tile_pool(name="cc", bufs=4, space="DRAM") as cc_pool,
        ):
            wt_sb = singles.tile([P, K_TILES, VC], FP8)
            adj_sb = singles.tile([P, 1], F32)

            def emit_pass2_mtile(stage_m, lse, col, m, tail=False):
                nc.vector.tensor_scalar(
                    out=stage_m[:, :],
                    in0=stage_m[:, :],
                    scalar1=lse[:, col : col + 1],
                    scalar2=None,
                    op0=mybir.AluOpType.subtract,
                )
                nc.sync.dma_start(
                    out=out[m * P : (m + 1) * P, :], in_=stage_m[:, :]
                )

            # pass2 backlog: one entry per finished block awaiting its
            # all-reduced sums; drained 1 m-tile per seam with >= 4
            # m-tiles of lag (~52us >> the ~33.5us trigger+AllReduce
            # latency, so an emitted subtract never parks the in-order
            # DVE on a CC wait and the psum drains behind it never
            # starve the PE).
            backlog = []

            def drain_backlog(budget, min_age, tail=False):
                emitted = 0
                while backlog and emitted < budget:
                    e = backlog[0]
                    if e["age"] < min_age:
                        break
                    if e["lse"] is None:
                        lse = stats_pool.tile([P, e["n"]], F32, name="lse")
                        nc.scalar.activation(
                            out=lse,
                            in_=e["gsums"],
                            func=mybir.ActivationFunctionType.Ln,
                        )
                        e["lse"] = lse
                    col = e["done"]
                    emit_pass2_mtile(
                        e["tiles"][col], e["lse"], col, e["m0"] + col,
                        tail=tail,
                    )
                    e["done"] += 1
                    emitted += 1
                    if e["done"] == e["n"]:
                        backlog.pop(0)

            m0 = 0
            for blk, blk_n in enumerate(BLOCKS):
                stage_tiles = []
                lsums = stats_pool.tile([P, blk_n], F32, name="lsums")
                for mb in range(blk_n):
                    m = m0 + mb
                    ht = hts_pool.tile([P, K_TILES, P], FP8)
                    # gpsimd trigger: a nearly-empty in-order queue, so the
                    # prefetch fires promptly, and the transfer stays off
                    # the sync DMA queue that carries the 1.6MB output tiles
                    nc.gpsimd.dma_start(
                        out=ht, in_=hT_r[:, :, m * P : (m + 1) * P]
                    )
                    if blk == 0 and mb == 0:
                        # W prefetch in vocab-chunk order: the columns the
                        # first matmuls need arrive first.
                        for goff, gw in GROUPS:
                            for k in range(K_TILES):
                                nc.sync.dma_start(
                                    out=wt_sb[:, k, goff : goff + gw],
                                    in_=wT_r[:, k, goff : goff + gw],
                                )
                        nc.sync.dma_start(out=adj_sb, in_=adj)
                    stage_m = stage_pool.tile([P, VC], BF16, tag="stage")
                    sums_acc = stats_pool.tile([P, len(GROUPS)], F32)
                    for g, (goff, gw) in enumerate(GROUPS):
                        ps = psum_pool.tile([P, GW], F32, tag="ps")
                        for kp in range(K_PAIRS):
                            for j in range(0, gw, 512):
                                cs = min(512, gw - j)
                                nc.tensor.matmul(
                                    out=ps[:, j : j + cs],
                                    lhsT=ht[:, 2 * kp : 2 * kp + 2, :],
                                    rhs=wt_sb[
                                        :,
                                        2 * kp : 2 * kp + 2,
                                        goff + j : goff + j + cs,
                                    ],
                                    start=(kp == 0),
                                    stop=(kp == K_PAIRS - 1),
                                    perf_mode=mybir.MatmulPerfMode.DoubleRow,
                                )
                        # single DVE read frees the psum tile; exp reads
                        # the staged bf16 copy on the scalar engine. High
                        # priority: the list scheduler must never place a
                        # pass2 subtract burst ahead of a psum drain the
                        # PE is waiting on.
                        nc.vector.tensor_scalar(
                            out=stage_m[:, goff : goff + gw],
                            in0=ps[:, :gw],
                            scalar1=1.0 / W_SCALE,
                            scalar2=None,
                            op0=mybir.AluOpType.mult,
                        )
                        exp_scr = scratch_pool.tile([P, GW], BF16)
                        nc.scalar.activation(
                            out=exp_scr[:, :gw],
                            in_=stage_m[:, goff : goff + gw],
                            func=mybir.ActivationFunctionType.Exp,
                            accum_out=sums_acc[:, g : g + 1],
                        )
                    red = stats_pool.tile([P, 1], F32, name="red")
                    nc.vector.tensor_reduce(
                        out=red,
                        in_=sums_acc,
                        axis=mybir.AxisListType.X,
                        op=mybir.AluOpType.add,
                    )
                    nc.vector.tensor_add(
                        out=lsums[:, mb : mb + 1], in0=red, in1=adj_sb
                    )
                    stage_tiles.append(stage_m)

                    for e in backlog:
                        e["age"] += 1
                    drain_backlog(budget=1, min_age=4)

                cc_in = cc_pool.tile([P, blk_n], F32, tag="cc_in")
                cc_out = cc_pool.tile([P, blk_n], F32, tag="cc_out")
                nc.gpsimd.dma_start(out=cc_in[:, :], in_=lsums[:, :])
                nc.gpsimd.collective_compute(
                    "AllReduce",
                    mybir.AluOpType.add,
                    replica_groups=[list(range(N_CORES))],
                    ins=[cc_in[:, :].opt()],
                    outs=[cc_out[:, :].opt()],
                )
                gsums = stats_pool.tile([P, blk_n], F32, name="gsums")
                nc.gpsimd.dma_start(out=gsums[:, :], in_=cc_out[:, :])

                backlog.append(
                    {
                        "tiles": stage_tiles,
                        "gsums": gsums,
                        "m0": m0,
                        "n": blk_n,
                        "done": 0,
                        "lse": None,
                        "age": 0,
                    }
                )
                m0 += blk_n
            drain_backlog(budget=1000, min_age=0, tail=True)
    nc.compile()
    return nc


def _prep_inputs(hidden_states, W):
    """Host-side shard + transpose + cast. Returns per-core input maps."""
    hflat = np.asarray(hidden_states, dtype=np.float32).reshape(T, D)
    hT = np.ascontiguousarray(hflat.T).astype(NP_FP8)

    W = np.asarray(W, dtype=np.float32)
    in_maps = []
    for c in range(N_CORES):
        lo, hi = c * VC, (c + 1) * VC
        shard = W[lo : min(hi, V)]
        n_pad = VC - shard.shape[0]
        wT_c = np.zeros((D, VC), dtype=NP_FP8)
        wT_c[:, : shard.shape[0]] = (shard.T * W_SCALE).astype(NP_FP8)
        adj_c = np.full((P, 1), -float(n_pad), dtype=np.float32)
        in_maps.append({"hT": hT, "wT": wT_c, "adj": adj_c})
    return in_maps


def kernel(hidden_states, W):
    global LAST_RESULT
    in_maps = _prep_inputs(hidden_states, W)
    nc = build_nc()
    trace = os.environ.get("LMHEAD_TRACE", "0") == "1"
    res = run_bass_kernel_spmd(
        nc, in_maps, list(range(N_CORES)), trace=trace
    )
    LAST_RESULT = res
    parts = [
        np.asarray(res.results[c]["out"]).astype(np.float32)
        for c in range(N_CORES)
    ]
    full = np.concatenate(parts, axis=1)[:, :V]
    return np.ascontiguousarray(full.reshape(B, S, V).astype(np.float32))
